# revision 42
# baseline (speedup 1.0000x reference)
"""Trainium2 Bass kernel for nn_Attention_4449586119407.

GQA attention layer (B=2, L=2048, D=2048, 32 Q heads / 8 KV heads, RoPE,
causal) sharded over 8 NeuronCores: data-parallel over batch (2) x
tensor-parallel over heads (4 groups of 8 Q heads / 2 KV heads).
wq/wk/wv column-sharded, wo row-sharded; the wo partial sums are reduced
on the host.

Device-side layout (per core):
  - All matmuls contract over the SBUF partition dim.  x is pre-transposed
    on the host (xT [D, L]) so QKV projections produce channel-major
    qT/kT [c, l] directly.
  - RoPE pairs are made partition-contiguous by permuting wq/wk rows on the
    host (per head: even rows then odd rows -> [te(32), to(32)] blocks).
    The rotation partner (partition swap te<->to) is produced with 4 small
    SBUF->SBUF DMAs; the rotation itself is 3 full-width DVE ops using
    host-precomputed cos / signed-sin maps.
  - Scores are computed transposed (S[j, i] = k . q) so the softmaxed tile
    can feed the P@V matmul directly as the stationary operand.  Softmax
    skips the max-subtraction (logits are O(5) here) and gets the
    denominator for free as a 65th "ones" column of V.
  - Causality is structural: only lower-triangle j-blocks are computed and
    the 128x128 diagonal blocks get the (transposed) mask block added in
    PSUM before the exp.
"""

import numpy as np

B, L, D = 2, 2048, 2048
NH, NKV, HD = 32, 8, 64
SCALE = HD ** -0.5
NCORES = 8
F32 = np.float32

_CACHE = {}


def _build_nc(debug=False):
    from contextlib import ExitStack

    import concourse.tile as tile
    from concourse import bacc, mybir

    f32 = mybir.dt.float32
    f32r = mybir.dt.float32r
    f16 = mybir.dt.float16
    AF = mybir.ActivationFunctionType
    ALU = mybir.AluOpType

    nc = bacc.Bacc("TRN2", target_bir_lowering=False, debug=False,
                   num_devices=NCORES)

    xT = nc.dram_tensor("xT", [D, L], f16, kind="ExternalInput").ap()
    wq_sb_d = nc.dram_tensor("wq_sb", [128, 8192], f16, kind="ExternalInput").ap()
    wk_sb_d = nc.dram_tensor("wk_sb", [128, 2048], f16, kind="ExternalInput").ap()
    wv_sb_d = nc.dram_tensor("wv_sb", [128, 2048], f16, kind="ExternalInput").ap()
    wo_sb_d = nc.dram_tensor("wo_sb", [128, 8192], f16, kind="ExternalInput").ap()
    cosm_d = nc.dram_tensor("cosm", [128, 2048], f32, kind="ExternalInput").ap()
    sinm2_d = nc.dram_tensor("sinm2", [128, 2048], f32, kind="ExternalInput").ap()
    maskT_d = nc.dram_tensor("maskT", [128, 128], f32, kind="ExternalInput").ap()
    ident_d = nc.dram_tensor("ident", [128, 128], f32, kind="ExternalInput").ap()
    y_d = nc.dram_tensor("y", [L, D], f32, kind="ExternalOutput").ap()
    if debug:
        dbg = {nm: nc.dram_tensor(f"dbg_{nm}", sh, dt, kind="ExternalOutput").ap()
               for nm, sh, dt in [
                   ("q00", [128, 512], f16), ("kz00", [128, 2048], f16),
                   ("vext0", [128, 2048], f16), ("p000", [128, 1024], f16),
                   ("attT00", [128, 512], f16), ("att0", [128, 512], f32), ("rcpb0", [64, 512], f32)]}

    with tile.TileContext(nc) as tc:
        with ExitStack() as ctx:
            singles = ctx.enter_context(tc.tile_pool(name="singles", bufs=1))
            xt_p = ctx.enter_context(tc.tile_pool(name="xt", bufs=24))
            maps_p = ctx.enter_context(tc.tile_pool(name="maps", bufs=4))
            qrot_p = ctx.enter_context(tc.tile_pool(name="qrot", bufs=16))
            ptile_p = ctx.enter_context(tc.tile_pool(name="pt", bufs=3))
            tmp_p = ctx.enter_context(tc.tile_pool(name="tmp", bufs=3))
            attT_p = ctx.enter_context(tc.tile_pool(name="attT", bufs=12))
            rcpb_p = ctx.enter_context(tc.tile_pool(name="rcpb", bufs=2))
            ysb_p = ctx.enter_context(tc.tile_pool(name="ysb", bufs=3))
            pj_ps = ctx.enter_context(tc.tile_pool(name="pj", bufs=2, space="PSUM"))
            sc_ps = ctx.enter_context(tc.tile_pool(name="sc", bufs=2, space="PSUM"))
            at_ps = ctx.enter_context(tc.tile_pool(name="at", bufs=2, space="PSUM"))

            # ---- HAM warmup: keep PE busy while the first DMAs land ----
            warm_sb = singles.tile([128, 128], f16)
            nc.vector.memset(warm_sb[:], 0.0)
            warm_ps = pj_ps.tile([64, 64], f32, tag="pj", name="warm_ps")
            for _ in range(80):
                nc.tensor.matmul(warm_ps[:], warm_sb[:, 0:64], warm_sb[:, 0:64],
                                 start=True, stop=True)

            # ---- resident constants (wq cb0 first; bulk after xt chunk 0) ----
            wq_sb = singles.tile([128, 8192], f16)
            nc.sync.dma_start(wq_sb[:, 0:2048], wq_sb_d[:, 0:2048])
            wk_sb = singles.tile([128, 2048], f16)
            wv_sb = singles.tile([128, 2048], f16)
            mask_sb = singles.tile([128, 128], f32)
            ident_sb = singles.tile([128, 128], f32)
            wo_sb = singles.tile([128, 8192], f16)

            def load_weights_bulk():
                for cb in range(1, 4):
                    nc.sync.dma_start(wq_sb[:, cb * 2048:(cb + 1) * 2048],
                                      wq_sb_d[:, cb * 2048:(cb + 1) * 2048])
                nc.sync.dma_start(wk_sb[:], wk_sb_d[:])
                nc.sync.dma_start(wv_sb[:], wv_sb_d[:])
                nc.sync.dma_start(mask_sb[:], maskT_d[:])
                nc.sync.dma_start(ident_sb[:], ident_d[:])
                nc.sync.dma_start(wo_sb[:], wo_sb_d[:])
            # kz[kv][half]: rotated k for kv head, in partition half `half`,
            # other half zero -> K=128 scores matmuls with full partitions
            kz = [[singles.tile([128, 2048], f16, name=f"kz{kv}{hf}")
                   for hf in range(2)] for kv in range(2)]
            for kv in range(2):
                for hf in range(2):
                    nc.vector.memset(kz[kv][hf][:], 0.0)
            # per jb: [v(64) | ones(1) | pad(63)] -> full 128-col stationary
            vext = [singles.tile([128, 2048], f16, name=f"vext{kv}")
                    for kv in range(2)]
            for kv in range(2):
                nc.vector.memset(vext[kv][:], 1.0)

            def diag_off(jb, lc):
                """col offset of the diagonal 128-block inside chunk lc, or None"""
                od = 128 * jb - 512 * lc
                return od if 0 <= od < 512 else None

            q_tiles = {}

            def load_xt(lc):
                lsl = slice(lc * 512, (lc + 1) * 512)
                xt = []
                for db in range(16):
                    t = xt_p.tile([128, 512], f16, tag="xt", name=f"xt{db}")
                    nc.sync.dma_start(t[:], xT[db * 128:(db + 1) * 128, lsl])
                    xt.append(t)
                cos_t = maps_p.tile([128, 512], f32, tag="cos", name="cos_t")
                nc.sync.dma_start(cos_t[:], cosm_d[:, lsl])
                sin_t = maps_p.tile([128, 512], f32, tag="sin", name="sin_t")
                nc.sync.dma_start(sin_t[:], sinm2_d[:, lsl])
                return xt, cos_t, sin_t

            def do_proj(lc, loaded):
                lsl = slice(lc * 512, (lc + 1) * 512)
                xt, cos_t, sin_t = loaded

                def rope(ps, dest):
                    """rotate [128,512] psum block into dest (SBUF)."""
                    t2 = tmp_p.tile([128, 512], f32, tag="t2", bufs=2, name="t2")
                    nc.vector.tensor_tensor(t2[:], ps[:], sin_t[:], op=ALU.mult)
                    qsw = tmp_p.tile([128, 512], f32, tag="qsw", name="qsw")
                    for g, srcp in enumerate((32, 0, 96, 64)):
                        nc.sync.dma_start(qsw[g * 32:(g + 1) * 32, :],
                                          t2[srcp:srcp + 32, :])
                    t3 = tmp_p.tile([128, 512], f32, tag="t3", bufs=2, name="t3")
                    nc.vector.tensor_tensor(t3[:], ps[:], cos_t[:], op=ALU.mult)
                    nc.vector.tensor_tensor(dest[:], t3[:], qsw[:], op=ALU.add)

                # q projection + RoPE
                for cb in range(4):
                    ps = pj_ps.tile([128, 512], f32, tag="pj", name="ps_q")
                    for db in range(16):
                        nc.tensor.matmul(
                            ps[:], wq_sb[:, (cb * 16 + db) * 128:(cb * 16 + db + 1) * 128],
                            xt[db][:], start=(db == 0), stop=(db == 15))
                    qt = qrot_p.tile([128, 512], f16, name="qt")
                    rope(ps, qt)
                    q_tiles[(cb, lc)] = qt

                # k projection + RoPE + zero-padded scatter
                ps = pj_ps.tile([128, 512], f32, tag="pj", name="ps_k")
                for db in range(16):
                    nc.tensor.matmul(
                        ps[:], wk_sb[:, db * 128:(db + 1) * 128],
                        xt[db][:], start=(db == 0), stop=(db == 15))
                kraw = tmp_p.tile([128, 512], f16, tag="kraw", bufs=2, name="kraw")
                rope(ps, kraw)
                for kv in range(2):
                    nc.sync.dma_start(kz[kv][0][0:64, lsl],
                                      kraw[kv * 64:kv * 64 + 64, :])
                    nc.sync.dma_start(kz[kv][1][64:128, lsl],
                                      kraw[kv * 64:kv * 64 + 64, :])

                # v projection (transposed) then PE-transpose per block
                vt = tmp_p.tile([128, 512], f32, tag="vt", bufs=2, name="vt")
                ps = pj_ps.tile([128, 512], f32, tag="pj", name="ps_v")
                for db in range(16):
                    nc.tensor.matmul(
                        ps[:], wv_sb[:, db * 128:(db + 1) * 128],
                        xt[db][:], start=(db == 0), stop=(db == 15))
                nc.vector.tensor_copy(vt[:], ps[:])
                for j in range(4):
                    jb = 4 * lc + j
                    ps = pj_ps.tile([128, 128], f32, tag="pj", name="ps_t")
                    nc.tensor.transpose(ps[:], vt[:, j * 128:(j + 1) * 128],
                                        ident_sb[:])
                    for kv in range(2):
                        nc.vector.tensor_copy(
                            vext[kv][:, jb * 128:jb * 128 + 64],
                            ps[:, kv * 64:kv * 64 + 64])

            def do_att(lc, pairs=(0, 1, 2, 3)):
                njb = 4 * lc + 4
                for t in pairs:
                    heads = (2 * t, 2 * t + 1)
                    aps_ = [at_ps.tile([128, 512], f32, tag="at", name=f"at{e}")
                            for e in range(2)]
                    P = None
                    for jb in range(njb):
                        o = max(0, 128 * jb - 512 * lc)
                        S = sc_ps.tile([128, 1024], f32, tag="sc", name="S")
                        for e, h in enumerate(heads):
                            kt = kz[h // 4][h % 2]
                            nc.tensor.matmul(
                                S[:, e * 512 + o:(e + 1) * 512],
                                kt[:, jb * 128:(jb + 1) * 128],
                                q_tiles[(h // 2, lc)][:, o:512],
                                start=True, stop=True)
                        od = diag_off(jb, lc)
                        if od is not None:
                            s3 = S.rearrange("p (e c) -> p e c", e=2)[:, :, od:od + 128]
                            m3 = mask_sb[:].unsqueeze(1).broadcast_to([128, 2, 128])
                            nc.vector.tensor_tensor(s3, s3, m3, op=ALU.add)
                        P = ptile_p.tile([128, 1024], f16, name="P")
                        if o == 0:
                            nc.scalar.activation(P[:], S[:], AF.Exp)
                        else:
                            for e in range(2):
                                nc.scalar.activation(
                                    P[:, e * 512 + o:(e + 1) * 512],
                                    S[:, e * 512 + o:(e + 1) * 512], AF.Exp)
                        for e, h in enumerate(heads):
                            kv = h // 4
                            nc.tensor.matmul(
                                aps_[e][:, o:512],
                                vext[kv][:, jb * 128:jb * 128 + 128],
                                P[:, e * 512 + o:(e + 1) * 512],
                                start=(jb == 0), stop=(jb == njb - 1),
                                skip_group_check=True)
                    if debug and lc == 0 and t == 0:
                        nc.sync.dma_start(dbg["p000"][:], P[:])
                    attT = attT_p.tile([128, 512], f16, name="attT")
                    q_tiles[("attT", lc, t)] = attT
                    if debug and lc == 0 and t == 0:
                        dbga = tmp_p.tile([128, 512], f32, tag="dbga", bufs=1,
                                          name="dbga")
                        nc.vector.tensor_copy(dbga[:], aps_[0][:])
                        nc.sync.dma_start(dbg["att0"][:], dbga[:])
                    for e, h in enumerate(heads):
                        den = rcpb_p.tile([1, 512], f32, tag="den", bufs=2,
                                          name="den")
                        nc.vector.tensor_copy(den[:], aps_[e][64:65, :])
                        rcpb = rcpb_p.tile([64, 512], f32, name="rcpb")
                        nc.vector.reciprocal_approx_fast(out=rcpb[0:1, :], in_=den[:])
                        nc.gpsimd.partition_broadcast(rcpb[:], rcpb[0:1, :])
                        if debug and lc == 0 and t == 0 and e == 0:
                            nc.sync.dma_start(dbg["rcpb0"][:], rcpb[:])
                        nc.vector.tensor_tensor(
                            attT[64 * e:64 * e + 64, :], aps_[e][0:64, :],
                            rcpb[:], op=ALU.mult)

            def do_outproj(lc):
                for lb in range(4):
                    for mc in range(4):
                        ps = pj_ps.tile([128, 512], f32, tag="pj", name="ps_o")
                        for cb in range(4):
                            nc.tensor.matmul(
                                ps[:],
                                q_tiles[("attT", lc, cb)][:, lb * 128:(lb + 1) * 128],
                                wo_sb[:, cb * 2048 + mc * 512:cb * 2048 + (mc + 1) * 512],
                                start=(cb == 0), stop=(cb == 3))
                        ysb = ysb_p.tile([128, 512], f32, name="ysb")
                        nc.vector.tensor_copy(ysb[:], ps[:])
                        nc.sync.dma_start(
                            y_d[lc * 512 + lb * 128:lc * 512 + (lb + 1) * 128,
                                mc * 512:(mc + 1) * 512], ysb[:])

            ld = load_xt(0)
            load_weights_bulk()
            do_proj(0, ld)
            ld = load_xt(1)
            do_att(0, pairs=(0, 1))
            do_proj(1, ld)
            ld = load_xt(2)
            do_att(1)
            do_proj(2, ld)
            ld = load_xt(3)
            do_outproj(1)
            do_att(2)
            do_proj(3, ld)
            do_outproj(2)
            do_att(3)
            do_outproj(3)
            do_att(0, pairs=(2, 3))
            do_outproj(0)

            if debug:
                nc.sync.dma_start(dbg["q00"][:], q_tiles[(0, 0)][:])
                nc.sync.dma_start(dbg["kz00"][:], kz[0][0][:])
                nc.sync.dma_start(dbg["vext0"][:], vext[0][:])
                nc.sync.dma_start(dbg["attT00"][:], q_tiles[("attT", 0, 0)][:])

    nc.compile()
    return nc


def _perm64(w):
    return np.concatenate([w[0::2], w[1::2]], axis=0)


def _prep_core_inputs(core, x, wq, wk, wv, wo, fc, fs, mask, xT_cache):
    b, g = divmod(core, 4)
    hq0 = 8 * g
    if b not in xT_cache:
        xT_cache[b] = np.ascontiguousarray(x[b].T, dtype=np.float16)
    xT = xT_cache[b]

    wq_s = (wq[hq0 * 64:(hq0 + 8) * 64] * SCALE).astype(F32)
    wq_p = np.concatenate([_perm64(wq_s[h * 64:(h + 1) * 64]) for h in range(8)], 0)
    wqT = wq_p.T  # [D, 512]
    # wq_sb[p, (cb*16+db)*128 + c] = wqT[db*128+p, cb*128+c]
    wq_sb = np.ascontiguousarray(
        wqT.reshape(16, 128, 4, 128).transpose(1, 2, 0, 3).reshape(128, 8192),
        dtype=np.float16)

    wk_s = wk[2 * g * 64:(2 * g + 2) * 64]
    wk_p = np.concatenate([_perm64(wk_s[h * 64:(h + 1) * 64]) for h in range(2)], 0)
    wkT = wk_p.T  # [D, 128]
    wk_sb = np.ascontiguousarray(
        wkT.reshape(16, 128, 128).transpose(1, 0, 2).reshape(128, 2048),
        dtype=np.float16)

    wvT = wv[2 * g * 64:(2 * g + 2) * 64].T  # [D, 128]
    wv_sb = np.ascontiguousarray(
        wvT.reshape(16, 128, 128).transpose(1, 0, 2).reshape(128, 2048),
        dtype=np.float16)

    woT = wo[:, hq0 * 64:(hq0 + 8) * 64].T  # [512, D]
    wo_sb = np.ascontiguousarray(
        woT.reshape(4, 128, 4, 512).transpose(1, 0, 2, 3).reshape(128, 8192),
        dtype=np.float16)

    cosT = np.tile(fc.T, (4, 1)).astype(F32)  # [128, L]
    sinT = np.tile(fs.T, (4, 1)).astype(F32)
    sgn = np.ones((128, 1), F32)
    sgn[32:64] = -1
    sgn[96:128] = -1
    sinm2 = np.ascontiguousarray(sinT * sgn)

    maskT = np.ascontiguousarray(mask[0, 0, :128, :128].T, dtype=F32)

    return {"xT": xT, "wq_sb": wq_sb, "wk_sb": wk_sb, "wv_sb": wv_sb,
            "wo_sb": wo_sb, "cosm": np.ascontiguousarray(cosT),
            "sinm2": sinm2, "maskT": maskT,
            "ident": np.eye(128, dtype=F32)}


def kernel(x, wq, wk, wv, wo, freqs_cos, freqs_sin, mask):
    from concourse import bass_utils

    if "nc" not in _CACHE:
        _CACHE["nc"] = _build_nc()
    nc = _CACHE["nc"]

    x = np.asarray(x, F32)
    xT_cache = {}
    in_maps = [
        _prep_core_inputs(c, x, np.asarray(wq, F32), np.asarray(wk, F32),
                          np.asarray(wv, F32), np.asarray(wo, F32),
                          np.asarray(freqs_cos, F32), np.asarray(freqs_sin, F32),
                          np.asarray(mask, F32), xT_cache)
        for c in range(NCORES)
    ]
    res = bass_utils.run_bass_kernel_spmd(nc, in_maps, core_ids=list(range(NCORES)))
    out = np.zeros((B, L, D), F32)
    for c in range(NCORES):
        out[c // 4] += res.results[c]["y"]
    return out


# revision 44
# speedup vs baseline: 1.0781x; 1.0781x over previous
"""Trainium2 Bass kernel for nn_Attention_4449586119407.

GQA attention layer (B=2, L=2048, D=2048, 32 Q heads / 8 KV heads, RoPE,
causal) sharded over 8 NeuronCores: data-parallel over batch (2) x
tensor-parallel over heads (4 groups of 8 Q heads / 2 KV heads).
wq/wk/wv column-sharded, wo row-sharded; the wo partial sums are reduced
on the host.

Device-side layout (per core):
  - All matmuls contract over the SBUF partition dim.  x is pre-transposed
    on the host (xT [D, L]) so QKV projections produce channel-major
    qT/kT [c, l] directly.
  - RoPE pairs are made partition-contiguous by permuting wq/wk rows on the
    host (per head: even rows then odd rows -> [te(32), to(32)] blocks).
    The rotation partner (partition swap te<->to) is produced with 4 small
    SBUF->SBUF DMAs; the rotation itself is 3 full-width DVE ops using
    host-precomputed cos / signed-sin maps.
  - Scores are computed transposed (S[j, i] = k . q) so the softmaxed tile
    can feed the P@V matmul directly as the stationary operand.  Softmax
    skips the max-subtraction (logits are O(5) here) and gets the
    denominator for free as a 65th "ones" column of V.
  - Causality is structural: only lower-triangle j-blocks are computed and
    the 128x128 diagonal blocks get the (transposed) mask block added in
    PSUM before the exp.
"""

import numpy as np

B, L, D = 2, 2048, 2048
NH, NKV, HD = 32, 8, 64
SCALE = HD ** -0.5
NCORES = 8
F32 = np.float32

_CACHE = {}


def _build_nc(debug=False):
    from contextlib import ExitStack

    import concourse.tile as tile
    from concourse import bacc, mybir

    f32 = mybir.dt.float32
    f32r = mybir.dt.float32r
    f16 = mybir.dt.float16
    AF = mybir.ActivationFunctionType
    ALU = mybir.AluOpType

    nc = bacc.Bacc("TRN2", target_bir_lowering=False, debug=False,
                   num_devices=NCORES)

    xT = nc.dram_tensor("xT", [D, L], f16, kind="ExternalInput").ap()
    wq_sb_d = nc.dram_tensor("wq_sb", [128, 8192], f16, kind="ExternalInput").ap()
    wk_sb_d = nc.dram_tensor("wk_sb", [128, 2048], f16, kind="ExternalInput").ap()
    wv_sb_d = nc.dram_tensor("wv_sb", [128, 2048], f16, kind="ExternalInput").ap()
    wo_sb_d = nc.dram_tensor("wo_sb", [128, 8192], f16, kind="ExternalInput").ap()
    cosm_d = nc.dram_tensor("cosm", [128, 2048], f32, kind="ExternalInput").ap()
    sinm2_d = nc.dram_tensor("sinm2", [128, 2048], f32, kind="ExternalInput").ap()
    maskT_d = nc.dram_tensor("maskT", [128, 128], f32, kind="ExternalInput").ap()
    ident_d = nc.dram_tensor("ident", [128, 128], f32, kind="ExternalInput").ap()
    y_d = nc.dram_tensor("y", [L, D], f32, kind="ExternalOutput").ap()
    if debug:
        dbg = {nm: nc.dram_tensor(f"dbg_{nm}", sh, dt, kind="ExternalOutput").ap()
               for nm, sh, dt in [
                   ("q00", [128, 512], f16), ("kz00", [128, 2048], f16),
                   ("vext0", [128, 2048], f16), ("p000", [128, 1024], f16),
                   ("attT00", [128, 512], f16), ("att0", [128, 512], f32), ("rcpb0", [64, 512], f32)]}

    with tile.TileContext(nc) as tc:
        with ExitStack() as ctx:
            singles = ctx.enter_context(tc.tile_pool(name="singles", bufs=1))
            xt_p = ctx.enter_context(tc.tile_pool(name="xt", bufs=24))
            maps_p = ctx.enter_context(tc.tile_pool(name="maps", bufs=4))
            qrot_p = ctx.enter_context(tc.tile_pool(name="qrot", bufs=8))
            ptile_p = ctx.enter_context(tc.tile_pool(name="pt", bufs=3))
            tmp_p = ctx.enter_context(tc.tile_pool(name="tmp", bufs=3))
            attT_p = ctx.enter_context(tc.tile_pool(name="attT", bufs=9))
            rcpb_p = ctx.enter_context(tc.tile_pool(name="rcpb", bufs=2))
            ysb_p = ctx.enter_context(tc.tile_pool(name="ysb", bufs=3))
            pj_ps = ctx.enter_context(tc.tile_pool(name="pj", bufs=2, space="PSUM"))
            sc_ps = ctx.enter_context(tc.tile_pool(name="sc", bufs=2, space="PSUM"))
            at_ps = ctx.enter_context(tc.tile_pool(name="at", bufs=2, space="PSUM"))

            # ---- HAM warmup: keep PE busy while the first DMAs land ----
            warm_sb = singles.tile([128, 128], f16)
            nc.vector.memset(warm_sb[:], 0.0)
            warm_ps = pj_ps.tile([64, 64], f32, tag="pj", name="warm_ps")
            for _ in range(150):
                nc.tensor.matmul(warm_ps[:], warm_sb[:, 0:64], warm_sb[:, 0:64],
                                 start=True, stop=True)

            # ---- resident constants (wq cb0 first; bulk after xt chunk 0) ----
            wq_sb = singles.tile([128, 8192], f16)
            nc.sync.dma_start(wq_sb[:, 0:2048], wq_sb_d[:, 0:2048])
            wk_sb = singles.tile([128, 2048], f16)
            wv_sb = singles.tile([128, 2048], f16)
            mask_sb = singles.tile([128, 128], f32)
            ident_sb = singles.tile([128, 128], f32)
            wo_sb = singles.tile([128, 8192], f16)

            def load_weights_bulk():
                for cb in range(1, 4):
                    nc.sync.dma_start(wq_sb[:, cb * 2048:(cb + 1) * 2048],
                                      wq_sb_d[:, cb * 2048:(cb + 1) * 2048])
                nc.sync.dma_start(wk_sb[:], wk_sb_d[:])
                nc.sync.dma_start(wv_sb[:], wv_sb_d[:])
                nc.sync.dma_start(mask_sb[:], maskT_d[:])
                nc.sync.dma_start(ident_sb[:], ident_d[:])
                nc.sync.dma_start(wo_sb[:], wo_sb_d[:])
            # kz[kv][half]: rotated k for kv head, in partition half `half`,
            # other half zero -> K=128 scores matmuls with full partitions
            kz = [[singles.tile([128, 2048], f16, name=f"kz{kv}{hf}")
                   for hf in range(2)] for kv in range(2)]
            for kv in range(2):
                for hf in range(2):
                    nc.vector.memset(kz[kv][hf][:], 0.0)
            # per jb: [v(64) | ones(1) | pad(63)] -> full 128-col stationary
            vext = [singles.tile([128, 2048], f16, name=f"vext{kv}")
                    for kv in range(2)]
            for kv in range(2):
                nc.vector.memset(vext[kv][:], 1.0)

            def diag_off(jb, lc):
                """col offset of the diagonal 128-block inside chunk lc, or None"""
                od = 128 * jb - 512 * lc
                return od if 0 <= od < 512 else None

            q_tiles = {}

            def load_xt(lc):
                lsl = slice(lc * 512, (lc + 1) * 512)
                xt = []
                for db in range(16):
                    t = xt_p.tile([128, 512], f16, tag="xt", name=f"xt{db}")
                    nc.sync.dma_start(t[:], xT[db * 128:(db + 1) * 128, lsl])
                    xt.append(t)
                cos_t = maps_p.tile([128, 512], f32, tag="cos", name="cos_t")
                nc.sync.dma_start(cos_t[:], cosm_d[:, lsl])
                sin_t = maps_p.tile([128, 512], f32, tag="sin", name="sin_t")
                nc.sync.dma_start(sin_t[:], sinm2_d[:, lsl])
                return xt, cos_t, sin_t

            def do_proj(lc, loaded):
                lsl = slice(lc * 512, (lc + 1) * 512)
                xt, cos_t, sin_t = loaded

                def rope(ps, dest):
                    """rotate [128,512] psum block into dest (SBUF)."""
                    t2 = tmp_p.tile([128, 512], f32, tag="t2", bufs=2, name="t2")
                    nc.vector.tensor_tensor(t2[:], ps[:], sin_t[:], op=ALU.mult)
                    qsw = tmp_p.tile([128, 512], f32, tag="qsw", name="qsw")
                    for g, srcp in enumerate((32, 0, 96, 64)):
                        nc.sync.dma_start(qsw[g * 32:(g + 1) * 32, :],
                                          t2[srcp:srcp + 32, :])
                    t3 = tmp_p.tile([128, 512], f32, tag="t3", bufs=2, name="t3")
                    nc.vector.tensor_tensor(t3[:], ps[:], cos_t[:], op=ALU.mult)
                    nc.vector.tensor_tensor(dest[:], t3[:], qsw[:], op=ALU.add)

                # q projection + RoPE
                for cb in range(4):
                    ps = pj_ps.tile([128, 512], f32, tag="pj", name="ps_q")
                    for db in range(16):
                        nc.tensor.matmul(
                            ps[:], wq_sb[:, (cb * 16 + db) * 128:(cb * 16 + db + 1) * 128],
                            xt[db][:], start=(db == 0), stop=(db == 15))
                    qt = qrot_p.tile([128, 512], f16, name="qt")
                    rope(ps, qt)
                    q_tiles[(cb, lc)] = qt

                # k projection + RoPE + zero-padded scatter
                ps = pj_ps.tile([128, 512], f32, tag="pj", name="ps_k")
                for db in range(16):
                    nc.tensor.matmul(
                        ps[:], wk_sb[:, db * 128:(db + 1) * 128],
                        xt[db][:], start=(db == 0), stop=(db == 15))
                kraw = tmp_p.tile([128, 512], f16, tag="kraw", bufs=2, name="kraw")
                rope(ps, kraw)
                for kv in range(2):
                    nc.sync.dma_start(kz[kv][0][0:64, lsl],
                                      kraw[kv * 64:kv * 64 + 64, :])
                    nc.sync.dma_start(kz[kv][1][64:128, lsl],
                                      kraw[kv * 64:kv * 64 + 64, :])

                # v projection (transposed) then PE-transpose per block
                vt = tmp_p.tile([128, 512], f32, tag="vt", bufs=2, name="vt")
                ps = pj_ps.tile([128, 512], f32, tag="pj", name="ps_v")
                for db in range(16):
                    nc.tensor.matmul(
                        ps[:], wv_sb[:, db * 128:(db + 1) * 128],
                        xt[db][:], start=(db == 0), stop=(db == 15))
                nc.vector.tensor_copy(vt[:], ps[:])
                for j in range(4):
                    jb = 4 * lc + j
                    ps = pj_ps.tile([128, 128], f32, tag="pj", name="ps_t")
                    nc.tensor.transpose(ps[:], vt[:, j * 128:(j + 1) * 128],
                                        ident_sb[:])
                    for kv in range(2):
                        nc.vector.tensor_copy(
                            vext[kv][:, jb * 128:jb * 128 + 64],
                            ps[:, kv * 64:kv * 64 + 64])

            def do_att(lc):
                njb = 4 * lc + 4
                for t in range(4):
                    heads = (2 * t, 2 * t + 1)
                    aps_ = [at_ps.tile([128, 512], f32, tag="at", name=f"at{e}")
                            for e in range(2)]
                    P = None
                    for jb in range(njb):
                        o = max(0, 128 * jb - 512 * lc)
                        S = sc_ps.tile([128, 1024], f32, tag="sc", name="S")
                        for e, h in enumerate(heads):
                            kt = kz[h // 4][h % 2]
                            nc.tensor.matmul(
                                S[:, e * 512 + o:(e + 1) * 512],
                                kt[:, jb * 128:(jb + 1) * 128],
                                q_tiles[(h // 2, lc)][:, o:512],
                                start=True, stop=True)
                        od = diag_off(jb, lc)
                        if od is not None:
                            s3 = S.rearrange("p (e c) -> p e c", e=2)[:, :, od:od + 128]
                            m3 = mask_sb[:].unsqueeze(1).broadcast_to([128, 2, 128])
                            nc.vector.tensor_tensor(s3, s3, m3, op=ALU.add)
                        P = ptile_p.tile([128, 1024], f16, name="P")
                        if o == 0:
                            nc.scalar.activation(P[:], S[:], AF.Exp)
                        else:
                            s3 = S.rearrange("p (e c) -> p e c", e=2)[:, :, o:512]
                            p3 = P.rearrange("p (e c) -> p e c", e=2)[:, :, o:512]
                            nc.scalar.activation(p3, s3, AF.Exp)
                        for e, h in enumerate(heads):
                            kv = h // 4
                            nc.tensor.matmul(
                                aps_[e][:, o:512],
                                vext[kv][:, jb * 128:jb * 128 + 128],
                                P[:, e * 512 + o:(e + 1) * 512],
                                start=(jb == 0), stop=(jb == njb - 1),
                                skip_group_check=True)
                    if debug and lc == 0 and t == 0:
                        nc.sync.dma_start(dbg["p000"][:], P[:])
                    attT = attT_p.tile([128, 512], f16, name="attT")
                    q_tiles[("attT", lc, t)] = attT
                    if debug and lc == 0 and t == 0:
                        dbga = tmp_p.tile([128, 512], f32, tag="dbga", bufs=1,
                                          name="dbga")
                        nc.vector.tensor_copy(dbga[:], aps_[0][:])
                        nc.sync.dma_start(dbg["att0"][:], dbga[:])
                    for e, h in enumerate(heads):
                        den = rcpb_p.tile([1, 512], f32, tag="den", bufs=2,
                                          name="den")
                        nc.vector.tensor_copy(den[:], aps_[e][64:65, :])
                        rcpb = rcpb_p.tile([64, 512], f32, name="rcpb")
                        nc.vector.reciprocal_approx_fast(out=rcpb[0:1, :], in_=den[:])
                        nc.gpsimd.partition_broadcast(rcpb[:], rcpb[0:1, :])
                        if debug and lc == 0 and t == 0 and e == 0:
                            nc.sync.dma_start(dbg["rcpb0"][:], rcpb[:])
                        nc.vector.tensor_tensor(
                            attT[64 * e:64 * e + 64, :], aps_[e][0:64, :],
                            rcpb[:], op=ALU.mult)

            def do_outproj(lc):
                for lb in range(4):
                    for mc in range(4):
                        ps = pj_ps.tile([128, 512], f32, tag="pj", name="ps_o")
                        for cb in range(4):
                            nc.tensor.matmul(
                                ps[:],
                                q_tiles[("attT", lc, cb)][:, lb * 128:(lb + 1) * 128],
                                wo_sb[:, cb * 2048 + mc * 512:cb * 2048 + (mc + 1) * 512],
                                start=(cb == 0), stop=(cb == 3))
                        ysb = ysb_p.tile([128, 512], f32, name="ysb")
                        nc.vector.tensor_copy(ysb[:], ps[:])
                        nc.sync.dma_start(
                            y_d[lc * 512 + lb * 128:lc * 512 + (lb + 1) * 128,
                                mc * 512:(mc + 1) * 512], ysb[:])

            ld = load_xt(0)
            load_weights_bulk()
            do_proj(0, ld)
            ld = load_xt(1)
            do_att(0)
            do_proj(1, ld)
            ld = load_xt(2)
            do_outproj(0)
            do_att(1)
            do_proj(2, ld)
            ld = load_xt(3)
            do_outproj(1)
            do_att(2)
            do_proj(3, ld)
            do_outproj(2)
            do_att(3)
            do_outproj(3)

            if debug:
                nc.sync.dma_start(dbg["q00"][:], q_tiles[(0, 0)][:])
                nc.sync.dma_start(dbg["kz00"][:], kz[0][0][:])
                nc.sync.dma_start(dbg["vext0"][:], vext[0][:])
                nc.sync.dma_start(dbg["attT00"][:], q_tiles[("attT", 0, 0)][:])

    nc.compile()
    return nc


def _perm64(w):
    return np.concatenate([w[0::2], w[1::2]], axis=0)


def _prep_core_inputs(core, x, wq, wk, wv, wo, fc, fs, mask, xT_cache):
    b, g = divmod(core, 4)
    hq0 = 8 * g
    if b not in xT_cache:
        xT_cache[b] = np.ascontiguousarray(x[b].T, dtype=np.float16)
    xT = xT_cache[b]

    wq_s = (wq[hq0 * 64:(hq0 + 8) * 64] * SCALE).astype(F32)
    wq_p = np.concatenate([_perm64(wq_s[h * 64:(h + 1) * 64]) for h in range(8)], 0)
    wqT = wq_p.T  # [D, 512]
    # wq_sb[p, (cb*16+db)*128 + c] = wqT[db*128+p, cb*128+c]
    wq_sb = np.ascontiguousarray(
        wqT.reshape(16, 128, 4, 128).transpose(1, 2, 0, 3).reshape(128, 8192),
        dtype=np.float16)

    wk_s = wk[2 * g * 64:(2 * g + 2) * 64]
    wk_p = np.concatenate([_perm64(wk_s[h * 64:(h + 1) * 64]) for h in range(2)], 0)
    wkT = wk_p.T  # [D, 128]
    wk_sb = np.ascontiguousarray(
        wkT.reshape(16, 128, 128).transpose(1, 0, 2).reshape(128, 2048),
        dtype=np.float16)

    wvT = wv[2 * g * 64:(2 * g + 2) * 64].T  # [D, 128]
    wv_sb = np.ascontiguousarray(
        wvT.reshape(16, 128, 128).transpose(1, 0, 2).reshape(128, 2048),
        dtype=np.float16)

    woT = wo[:, hq0 * 64:(hq0 + 8) * 64].T  # [512, D]
    wo_sb = np.ascontiguousarray(
        woT.reshape(4, 128, 4, 512).transpose(1, 0, 2, 3).reshape(128, 8192),
        dtype=np.float16)

    cosT = np.tile(fc.T, (4, 1)).astype(F32)  # [128, L]
    sinT = np.tile(fs.T, (4, 1)).astype(F32)
    sgn = np.ones((128, 1), F32)
    sgn[32:64] = -1
    sgn[96:128] = -1
    sinm2 = np.ascontiguousarray(sinT * sgn)

    maskT = np.ascontiguousarray(mask[0, 0, :128, :128].T, dtype=F32)

    return {"xT": xT, "wq_sb": wq_sb, "wk_sb": wk_sb, "wv_sb": wv_sb,
            "wo_sb": wo_sb, "cosm": np.ascontiguousarray(cosT),
            "sinm2": sinm2, "maskT": maskT,
            "ident": np.eye(128, dtype=F32)}


def kernel(x, wq, wk, wv, wo, freqs_cos, freqs_sin, mask):
    from concourse import bass_utils

    if "nc" not in _CACHE:
        _CACHE["nc"] = _build_nc()
    nc = _CACHE["nc"]

    x = np.asarray(x, F32)
    xT_cache = {}
    in_maps = [
        _prep_core_inputs(c, x, np.asarray(wq, F32), np.asarray(wk, F32),
                          np.asarray(wv, F32), np.asarray(wo, F32),
                          np.asarray(freqs_cos, F32), np.asarray(freqs_sin, F32),
                          np.asarray(mask, F32), xT_cache)
        for c in range(NCORES)
    ]
    res = bass_utils.run_bass_kernel_spmd(nc, in_maps, core_ids=list(range(NCORES)))
    out = np.zeros((B, L, D), F32)
    for c in range(NCORES):
        out[c // 4] += res.results[c]["y"]
    return out


# revision 45
# speedup vs baseline: 1.0832x; 1.0047x over previous
"""Trainium2 Bass kernel for nn_Attention_4449586119407.

GQA attention layer (B=2, L=2048, D=2048, 32 Q heads / 8 KV heads, RoPE,
causal) sharded over 8 NeuronCores: data-parallel over batch (2) x
tensor-parallel over heads (4 groups of 8 Q heads / 2 KV heads).
wq/wk/wv column-sharded, wo row-sharded; the wo partial sums are reduced
on the host.

Device-side layout (per core):
  - All matmuls contract over the SBUF partition dim.  x is pre-transposed
    on the host (xT [D, L]) so QKV projections produce channel-major
    qT/kT [c, l] directly.
  - RoPE pairs are made partition-contiguous by permuting wq/wk rows on the
    host (per head: even rows then odd rows -> [te(32), to(32)] blocks).
    The rotation partner (partition swap te<->to) is produced with 4 small
    SBUF->SBUF DMAs; the rotation itself is 3 full-width DVE ops using
    host-precomputed cos / signed-sin maps.
  - Scores are computed transposed (S[j, i] = k . q) so the softmaxed tile
    can feed the P@V matmul directly as the stationary operand.  Softmax
    skips the max-subtraction (logits are O(5) here) and gets the
    denominator for free as a 65th "ones" column of V.
  - Causality is structural: only lower-triangle j-blocks are computed and
    the 128x128 diagonal blocks get the (transposed) mask block added in
    PSUM before the exp.
"""

import numpy as np

B, L, D = 2, 2048, 2048
NH, NKV, HD = 32, 8, 64
SCALE = HD ** -0.5
NCORES = 8
F32 = np.float32

_CACHE = {}


def _build_nc(debug=False):
    from contextlib import ExitStack

    import concourse.tile as tile
    from concourse import bacc, mybir

    f32 = mybir.dt.float32
    f32r = mybir.dt.float32r
    f16 = mybir.dt.float16
    AF = mybir.ActivationFunctionType
    ALU = mybir.AluOpType

    nc = bacc.Bacc("TRN2", target_bir_lowering=False, debug=False,
                   num_devices=NCORES)

    xT = nc.dram_tensor("xT", [D, L], f16, kind="ExternalInput").ap()
    wq_sb_d = nc.dram_tensor("wq_sb", [128, 8192], f16, kind="ExternalInput").ap()
    wk_sb_d = nc.dram_tensor("wk_sb", [128, 2048], f16, kind="ExternalInput").ap()
    wv_sb_d = nc.dram_tensor("wv_sb", [128, 2048], f16, kind="ExternalInput").ap()
    wo_sb_d = nc.dram_tensor("wo_sb", [128, 8192], f16, kind="ExternalInput").ap()
    cosm_d = nc.dram_tensor("cosm", [128, 2048], f32, kind="ExternalInput").ap()
    sinm2_d = nc.dram_tensor("sinm2", [128, 2048], f32, kind="ExternalInput").ap()
    maskT_d = nc.dram_tensor("maskT", [128, 128], f32, kind="ExternalInput").ap()
    ident_d = nc.dram_tensor("ident", [128, 128], f32, kind="ExternalInput").ap()
    y_d = nc.dram_tensor("y", [L, D], f32, kind="ExternalOutput").ap()
    if debug:
        dbg = {nm: nc.dram_tensor(f"dbg_{nm}", sh, dt, kind="ExternalOutput").ap()
               for nm, sh, dt in [
                   ("q00", [128, 512], f16), ("kz00", [128, 2048], f16),
                   ("vext0", [128, 2048], f16), ("p000", [128, 1024], f16),
                   ("attT00", [128, 512], f16), ("att0", [128, 512], f32), ("rcpb0", [64, 512], f32)]}

    with tile.TileContext(nc) as tc:
        with ExitStack() as ctx:
            singles = ctx.enter_context(tc.tile_pool(name="singles", bufs=1))
            xt_p = ctx.enter_context(tc.tile_pool(name="xt", bufs=24))
            maps_p = ctx.enter_context(tc.tile_pool(name="maps", bufs=4))
            qrot_p = ctx.enter_context(tc.tile_pool(name="qrot", bufs=8))
            ptile_p = ctx.enter_context(tc.tile_pool(name="pt", bufs=3))
            tmp_p = ctx.enter_context(tc.tile_pool(name="tmp", bufs=3))
            attT_p = ctx.enter_context(tc.tile_pool(name="attT", bufs=9))
            rcpb_p = ctx.enter_context(tc.tile_pool(name="rcpb", bufs=2))
            ysb_p = ctx.enter_context(tc.tile_pool(name="ysb", bufs=3))
            pj_ps = ctx.enter_context(tc.tile_pool(name="pj", bufs=2, space="PSUM"))
            sc_ps = ctx.enter_context(tc.tile_pool(name="sc", bufs=2, space="PSUM"))
            at_ps = ctx.enter_context(tc.tile_pool(name="at", bufs=2, space="PSUM"))

            # ---- HAM warmup: keep PE busy while the first DMAs land ----
            warm_sb = singles.tile([128, 128], f16)
            nc.vector.memset(warm_sb[:], 0.0)
            warm_ps = pj_ps.tile([64, 64], f32, tag="pj", name="warm_ps")
            for _ in range(150):
                nc.tensor.matmul(warm_ps[:], warm_sb[:, 0:64], warm_sb[:, 0:64],
                                 start=True, stop=True)

            # ---- resident constants (wq cb0 first; bulk after xt chunk 0) ----
            wq_sb = singles.tile([128, 8192], f16)
            nc.sync.dma_start(wq_sb[:, 0:2048], wq_sb_d[:, 0:2048])
            wk_sb = singles.tile([128, 2048], f16)
            wv_sb = singles.tile([128, 2048], f16)
            mask_sb = singles.tile([128, 128], f32)
            ident_sb = singles.tile([128, 128], f32)
            wo_sb = singles.tile([128, 8192], f16)

            def load_weights_bulk():
                for cb in range(1, 4):
                    nc.sync.dma_start(wq_sb[:, cb * 2048:(cb + 1) * 2048],
                                      wq_sb_d[:, cb * 2048:(cb + 1) * 2048])
                nc.sync.dma_start(wk_sb[:], wk_sb_d[:])
                nc.sync.dma_start(wv_sb[:], wv_sb_d[:])
                nc.sync.dma_start(mask_sb[:], maskT_d[:])
                nc.sync.dma_start(ident_sb[:], ident_d[:])
            # kz[kv][half]: rotated k for kv head, in partition half `half`,
            # other half zero -> K=128 scores matmuls with full partitions
            kz = [[singles.tile([128, 2048], f16, name=f"kz{kv}{hf}")
                   for hf in range(2)] for kv in range(2)]
            for kv in range(2):
                for hf in range(2):
                    nc.vector.memset(kz[kv][hf][:], 0.0)
            # per jb: [v(64) | ones(1) | pad(63)] -> full 128-col stationary
            vext = [singles.tile([128, 2048], f16, name=f"vext{kv}")
                    for kv in range(2)]
            for kv in range(2):
                nc.vector.memset(vext[kv][:], 1.0)

            def diag_off(jb, lc):
                """col offset of the diagonal 128-block inside chunk lc, or None"""
                od = 128 * jb - 512 * lc
                return od if 0 <= od < 512 else None

            q_tiles = {}

            def load_xt(lc):
                lsl = slice(lc * 512, (lc + 1) * 512)
                xt = []
                for db in range(16):
                    t = xt_p.tile([128, 512], f16, tag="xt", name=f"xt{db}")
                    nc.sync.dma_start(t[:], xT[db * 128:(db + 1) * 128, lsl])
                    xt.append(t)
                cos_t = maps_p.tile([128, 512], f32, tag="cos", name="cos_t")
                nc.sync.dma_start(cos_t[:], cosm_d[:, lsl])
                sin_t = maps_p.tile([128, 512], f32, tag="sin", name="sin_t")
                nc.sync.dma_start(sin_t[:], sinm2_d[:, lsl])
                return xt, cos_t, sin_t

            def do_proj(lc, loaded):
                lsl = slice(lc * 512, (lc + 1) * 512)
                xt, cos_t, sin_t = loaded

                def rope(ps, dest):
                    """rotate [128,512] psum block into dest (SBUF)."""
                    t2 = tmp_p.tile([128, 512], f32, tag="t2", bufs=2, name="t2")
                    nc.vector.tensor_tensor(t2[:], ps[:], sin_t[:], op=ALU.mult)
                    qsw = tmp_p.tile([128, 512], f32, tag="qsw", name="qsw")
                    for g, srcp in enumerate((32, 0, 96, 64)):
                        nc.sync.dma_start(qsw[g * 32:(g + 1) * 32, :],
                                          t2[srcp:srcp + 32, :])
                    t3 = tmp_p.tile([128, 512], f32, tag="t3", bufs=2, name="t3")
                    nc.vector.tensor_tensor(t3[:], ps[:], cos_t[:], op=ALU.mult)
                    nc.vector.tensor_tensor(dest[:], t3[:], qsw[:], op=ALU.add)

                # q projection + RoPE
                for cb in range(4):
                    ps = pj_ps.tile([128, 512], f32, tag="pj", name="ps_q")
                    for db in range(16):
                        nc.tensor.matmul(
                            ps[:], wq_sb[:, (cb * 16 + db) * 128:(cb * 16 + db + 1) * 128],
                            xt[db][:], start=(db == 0), stop=(db == 15))
                    qt = qrot_p.tile([128, 512], f16, name="qt")
                    rope(ps, qt)
                    q_tiles[(cb, lc)] = qt

                # k projection + RoPE + zero-padded scatter
                ps = pj_ps.tile([128, 512], f32, tag="pj", name="ps_k")
                for db in range(16):
                    nc.tensor.matmul(
                        ps[:], wk_sb[:, db * 128:(db + 1) * 128],
                        xt[db][:], start=(db == 0), stop=(db == 15))
                kraw = tmp_p.tile([128, 512], f16, tag="kraw", bufs=2, name="kraw")
                rope(ps, kraw)
                for kv in range(2):
                    nc.sync.dma_start(kz[kv][0][0:64, lsl],
                                      kraw[kv * 64:kv * 64 + 64, :])
                    nc.sync.dma_start(kz[kv][1][64:128, lsl],
                                      kraw[kv * 64:kv * 64 + 64, :])

                # v projection (transposed) then PE-transpose per block
                vt = tmp_p.tile([128, 512], f32, tag="vt", bufs=2, name="vt")
                ps = pj_ps.tile([128, 512], f32, tag="pj", name="ps_v")
                for db in range(16):
                    nc.tensor.matmul(
                        ps[:], wv_sb[:, db * 128:(db + 1) * 128],
                        xt[db][:], start=(db == 0), stop=(db == 15))
                nc.vector.tensor_copy(vt[:], ps[:])
                for j in range(4):
                    jb = 4 * lc + j
                    ps = pj_ps.tile([128, 128], f32, tag="pj", name="ps_t")
                    nc.tensor.transpose(ps[:], vt[:, j * 128:(j + 1) * 128],
                                        ident_sb[:])
                    for kv in range(2):
                        nc.vector.tensor_copy(
                            vext[kv][:, jb * 128:jb * 128 + 64],
                            ps[:, kv * 64:kv * 64 + 64])

            def do_att(lc):
                njb = 4 * lc + 4
                for t in range(4):
                    heads = (2 * t, 2 * t + 1)
                    aps_ = [at_ps.tile([128, 512], f32, tag="at", name=f"at{e}")
                            for e in range(2)]
                    P = None
                    for jb in range(njb):
                        o = max(0, 128 * jb - 512 * lc)
                        S = sc_ps.tile([128, 1024], f32, tag="sc", name="S")
                        for e, h in enumerate(heads):
                            kt = kz[h // 4][h % 2]
                            nc.tensor.matmul(
                                S[:, e * 512 + o:(e + 1) * 512],
                                kt[:, jb * 128:(jb + 1) * 128],
                                q_tiles[(h // 2, lc)][:, o:512],
                                start=True, stop=True)
                        od = diag_off(jb, lc)
                        if od is not None:
                            s3 = S.rearrange("p (e c) -> p e c", e=2)[:, :, od:od + 128]
                            m3 = mask_sb[:].unsqueeze(1).broadcast_to([128, 2, 128])
                            nc.vector.tensor_tensor(s3, s3, m3, op=ALU.add)
                        P = ptile_p.tile([128, 1024], f16, name="P")
                        if o == 0:
                            nc.scalar.activation(P[:], S[:], AF.Exp)
                        else:
                            s3 = S.rearrange("p (e c) -> p e c", e=2)[:, :, o:512]
                            p3 = P.rearrange("p (e c) -> p e c", e=2)[:, :, o:512]
                            nc.scalar.activation(p3, s3, AF.Exp)
                        for e, h in enumerate(heads):
                            kv = h // 4
                            nc.tensor.matmul(
                                aps_[e][:, o:512],
                                vext[kv][:, jb * 128:jb * 128 + 128],
                                P[:, e * 512 + o:(e + 1) * 512],
                                start=(jb == 0), stop=(jb == njb - 1),
                                skip_group_check=True)
                    if debug and lc == 0 and t == 0:
                        nc.sync.dma_start(dbg["p000"][:], P[:])
                    attT = attT_p.tile([128, 512], f16, name="attT")
                    q_tiles[("attT", lc, t)] = attT
                    if debug and lc == 0 and t == 0:
                        dbga = tmp_p.tile([128, 512], f32, tag="dbga", bufs=1,
                                          name="dbga")
                        nc.vector.tensor_copy(dbga[:], aps_[0][:])
                        nc.sync.dma_start(dbg["att0"][:], dbga[:])
                    for e, h in enumerate(heads):
                        den = rcpb_p.tile([1, 512], f32, tag="den", bufs=2,
                                          name="den")
                        nc.vector.tensor_copy(den[:], aps_[e][64:65, :])
                        rcpb = rcpb_p.tile([64, 512], f32, name="rcpb")
                        nc.vector.reciprocal_approx_fast(out=rcpb[0:1, :], in_=den[:])
                        nc.gpsimd.partition_broadcast(rcpb[:], rcpb[0:1, :])
                        if debug and lc == 0 and t == 0 and e == 0:
                            nc.sync.dma_start(dbg["rcpb0"][:], rcpb[:])
                        nc.vector.tensor_tensor(
                            attT[64 * e:64 * e + 64, :], aps_[e][0:64, :],
                            rcpb[:], op=ALU.mult)

            def do_outproj(lc):
                for lb in range(4):
                    for mc in range(4):
                        ps = pj_ps.tile([128, 512], f32, tag="pj", name="ps_o")
                        for cb in range(4):
                            nc.tensor.matmul(
                                ps[:],
                                q_tiles[("attT", lc, cb)][:, lb * 128:(lb + 1) * 128],
                                wo_sb[:, cb * 2048 + mc * 512:cb * 2048 + (mc + 1) * 512],
                                start=(cb == 0), stop=(cb == 3))
                        ysb = ysb_p.tile([128, 512], f32, name="ysb")
                        nc.vector.tensor_copy(ysb[:], ps[:])
                        nc.sync.dma_start(
                            y_d[lc * 512 + lb * 128:lc * 512 + (lb + 1) * 128,
                                mc * 512:(mc + 1) * 512], ysb[:])

            ld = load_xt(0)
            load_weights_bulk()
            do_proj(0, ld)
            ld = load_xt(1)
            do_att(0)
            nc.sync.dma_start(wo_sb[:], wo_sb_d[:])
            do_proj(1, ld)
            ld = load_xt(2)
            do_outproj(0)
            do_att(1)
            do_proj(2, ld)
            ld = load_xt(3)
            do_outproj(1)
            do_att(2)
            do_proj(3, ld)
            do_outproj(2)
            do_att(3)
            do_outproj(3)

            if debug:
                nc.sync.dma_start(dbg["q00"][:], q_tiles[(0, 0)][:])
                nc.sync.dma_start(dbg["kz00"][:], kz[0][0][:])
                nc.sync.dma_start(dbg["vext0"][:], vext[0][:])
                nc.sync.dma_start(dbg["attT00"][:], q_tiles[("attT", 0, 0)][:])

    nc.compile()
    return nc


def _perm64(w):
    return np.concatenate([w[0::2], w[1::2]], axis=0)


def _prep_core_inputs(core, x, wq, wk, wv, wo, fc, fs, mask, xT_cache):
    b, g = divmod(core, 4)
    hq0 = 8 * g
    if b not in xT_cache:
        xT_cache[b] = np.ascontiguousarray(x[b].T, dtype=np.float16)
    xT = xT_cache[b]

    wq_s = (wq[hq0 * 64:(hq0 + 8) * 64] * SCALE).astype(F32)
    wq_p = np.concatenate([_perm64(wq_s[h * 64:(h + 1) * 64]) for h in range(8)], 0)
    wqT = wq_p.T  # [D, 512]
    # wq_sb[p, (cb*16+db)*128 + c] = wqT[db*128+p, cb*128+c]
    wq_sb = np.ascontiguousarray(
        wqT.reshape(16, 128, 4, 128).transpose(1, 2, 0, 3).reshape(128, 8192),
        dtype=np.float16)

    wk_s = wk[2 * g * 64:(2 * g + 2) * 64]
    wk_p = np.concatenate([_perm64(wk_s[h * 64:(h + 1) * 64]) for h in range(2)], 0)
    wkT = wk_p.T  # [D, 128]
    wk_sb = np.ascontiguousarray(
        wkT.reshape(16, 128, 128).transpose(1, 0, 2).reshape(128, 2048),
        dtype=np.float16)

    wvT = wv[2 * g * 64:(2 * g + 2) * 64].T  # [D, 128]
    wv_sb = np.ascontiguousarray(
        wvT.reshape(16, 128, 128).transpose(1, 0, 2).reshape(128, 2048),
        dtype=np.float16)

    woT = wo[:, hq0 * 64:(hq0 + 8) * 64].T  # [512, D]
    wo_sb = np.ascontiguousarray(
        woT.reshape(4, 128, 4, 512).transpose(1, 0, 2, 3).reshape(128, 8192),
        dtype=np.float16)

    cosT = np.tile(fc.T, (4, 1)).astype(F32)  # [128, L]
    sinT = np.tile(fs.T, (4, 1)).astype(F32)
    sgn = np.ones((128, 1), F32)
    sgn[32:64] = -1
    sgn[96:128] = -1
    sinm2 = np.ascontiguousarray(sinT * sgn)

    maskT = np.ascontiguousarray(mask[0, 0, :128, :128].T, dtype=F32)

    return {"xT": xT, "wq_sb": wq_sb, "wk_sb": wk_sb, "wv_sb": wv_sb,
            "wo_sb": wo_sb, "cosm": np.ascontiguousarray(cosT),
            "sinm2": sinm2, "maskT": maskT,
            "ident": np.eye(128, dtype=F32)}


def kernel(x, wq, wk, wv, wo, freqs_cos, freqs_sin, mask):
    from concourse import bass_utils

    if "nc" not in _CACHE:
        _CACHE["nc"] = _build_nc()
    nc = _CACHE["nc"]

    x = np.asarray(x, F32)
    xT_cache = {}
    in_maps = [
        _prep_core_inputs(c, x, np.asarray(wq, F32), np.asarray(wk, F32),
                          np.asarray(wv, F32), np.asarray(wo, F32),
                          np.asarray(freqs_cos, F32), np.asarray(freqs_sin, F32),
                          np.asarray(mask, F32), xT_cache)
        for c in range(NCORES)
    ]
    res = bass_utils.run_bass_kernel_spmd(nc, in_maps, core_ids=list(range(NCORES)))
    out = np.zeros((B, L, D), F32)
    for c in range(NCORES):
        out[c // 4] += res.results[c]["y"]
    return out


# revision 46
# speedup vs baseline: 1.1246x; 1.0382x over previous
"""Trainium2 Bass kernel for nn_Attention_4449586119407.

GQA attention layer (B=2, L=2048, D=2048, 32 Q heads / 8 KV heads, RoPE,
causal) sharded over 8 NeuronCores: data-parallel over batch (2) x
tensor-parallel over heads (4 groups of 8 Q heads / 2 KV heads).
wq/wk/wv column-sharded, wo row-sharded; the wo partial sums are reduced
on the host.

Device-side layout (per core):
  - All matmuls contract over the SBUF partition dim.  x is pre-transposed
    on the host (xT [D, L]) so QKV projections produce channel-major
    qT/kT [c, l] directly.
  - RoPE pairs are made partition-contiguous by permuting wq/wk rows on the
    host (per head: even rows then odd rows -> [te(32), to(32)] blocks).
    The rotation partner (partition swap te<->to) is produced with 4 small
    SBUF->SBUF DMAs; the rotation itself is 3 full-width DVE ops using
    host-precomputed cos / signed-sin maps.
  - Scores are computed transposed (S[j, i] = k . q) so the softmaxed tile
    can feed the P@V matmul directly as the stationary operand.  Softmax
    skips the max-subtraction (logits are O(5) here) and gets the
    denominator for free as a 65th "ones" column of V.
  - Causality is structural: only lower-triangle j-blocks are computed and
    the 128x128 diagonal blocks get the (transposed) mask block added in
    PSUM before the exp.
"""

import numpy as np

B, L, D = 2, 2048, 2048
NH, NKV, HD = 32, 8, 64
SCALE = HD ** -0.5
NCORES = 8
F32 = np.float32

_CACHE = {}


def _build_nc(debug=False):
    from contextlib import ExitStack

    import concourse.tile as tile
    from concourse import bacc, mybir

    f32 = mybir.dt.float32
    f32r = mybir.dt.float32r
    f16 = mybir.dt.float16
    AF = mybir.ActivationFunctionType
    ALU = mybir.AluOpType

    nc = bacc.Bacc("TRN2", target_bir_lowering=False, debug=False,
                   num_devices=NCORES)

    xT = nc.dram_tensor("xT", [D, L], f16, kind="ExternalInput").ap()
    wq_sb_d = nc.dram_tensor("wq_sb", [128, 8192], f16, kind="ExternalInput").ap()
    wk_sb_d = nc.dram_tensor("wk_sb", [128, 2048], f16, kind="ExternalInput").ap()
    wv_sb_d = nc.dram_tensor("wv_sb", [128, 2048], f16, kind="ExternalInput").ap()
    wo_sb_d = nc.dram_tensor("wo_sb", [128, 8192], f16, kind="ExternalInput").ap()
    cosm_d = nc.dram_tensor("cosm", [128, 2048], f32, kind="ExternalInput").ap()
    sinm2_d = nc.dram_tensor("sinm2", [128, 2048], f32, kind="ExternalInput").ap()
    maskT_d = nc.dram_tensor("maskT", [128, 128], f32, kind="ExternalInput").ap()
    ident_d = nc.dram_tensor("ident", [128, 128], f32, kind="ExternalInput").ap()
    y_d = nc.dram_tensor("y", [L, D], f32, kind="ExternalOutput").ap()
    if debug:
        dbg = {nm: nc.dram_tensor(f"dbg_{nm}", sh, dt, kind="ExternalOutput").ap()
               for nm, sh, dt in [
                   ("q00", [128, 512], f16), ("kz00", [128, 2048], f16),
                   ("vext0", [128, 2048], f16), ("p000", [128, 1024], f16),
                   ("attT00", [128, 512], f16), ("att0", [128, 512], f32), ("rcpb0", [64, 512], f32)]}

    with tile.TileContext(nc) as tc:
        with ExitStack() as ctx:
            singles = ctx.enter_context(tc.tile_pool(name="singles", bufs=1))
            xt_p = ctx.enter_context(tc.tile_pool(name="xt", bufs=24))
            maps_p = ctx.enter_context(tc.tile_pool(name="maps", bufs=4))
            qrot_p = ctx.enter_context(tc.tile_pool(name="qrot", bufs=8))
            ptile_p = ctx.enter_context(tc.tile_pool(name="pt", bufs=3))
            tmp_p = ctx.enter_context(tc.tile_pool(name="tmp", bufs=3))
            attT_p = ctx.enter_context(tc.tile_pool(name="attT", bufs=9))
            rcpb_p = ctx.enter_context(tc.tile_pool(name="rcpb", bufs=2))
            ysb_p = ctx.enter_context(tc.tile_pool(name="ysb", bufs=3))
            pj_ps = ctx.enter_context(tc.tile_pool(name="pj", bufs=2, space="PSUM"))
            sc_ps = ctx.enter_context(tc.tile_pool(name="sc", bufs=2, space="PSUM"))
            at_ps = ctx.enter_context(tc.tile_pool(name="at", bufs=2, space="PSUM"))

            # ---- HAM warmup: keep PE busy while the first DMAs land ----
            warm_sb = singles.tile([128, 128], f16)
            nc.vector.memset(warm_sb[:], 0.0)
            warm_ps = pj_ps.tile([64, 64], f32, tag="pj", name="warm_ps")
            for _ in range(150):
                nc.tensor.matmul(warm_ps[:], warm_sb[:, 0:64], warm_sb[:, 0:64],
                                 start=True, stop=True)

            # ---- resident constants (wq cb0 first; bulk after xt chunk 0) ----
            wq_sb = singles.tile([128, 8192], f16)
            nc.sync.dma_start(wq_sb[:, 0:2048], wq_sb_d[:, 0:2048])
            wk_sb = singles.tile([128, 2048], f16)
            wv_sb = singles.tile([128, 2048], f16)
            mask_sb = singles.tile([128, 128], f32)
            ident_sb = singles.tile([128, 128], f32)
            wo_sb = singles.tile([128, 8192], f16)

            def load_weights_bulk():
                for cb in range(1, 4):
                    nc.sync.dma_start(wq_sb[:, cb * 2048:(cb + 1) * 2048],
                                      wq_sb_d[:, cb * 2048:(cb + 1) * 2048])
                nc.sync.dma_start(wk_sb[:], wk_sb_d[:])
                nc.sync.dma_start(wv_sb[:], wv_sb_d[:])
                nc.sync.dma_start(mask_sb[:], maskT_d[:])
                nc.sync.dma_start(ident_sb[:], ident_d[:])
            # kz[kv][half]: rotated k for kv head, in partition half `half`,
            # other half zero -> K=128 scores matmuls with full partitions
            kz = [[singles.tile([128, 2048], f16, name=f"kz{kv}{hf}")
                   for hf in range(2)] for kv in range(2)]
            for kv in range(2):
                for hf in range(2):
                    nc.vector.memset(kz[kv][hf][:], 0.0)
            # per jb: [v(64) | ones(1) | pad(63)] -> full 128-col stationary
            vext = [singles.tile([128, 2048], f16, name=f"vext{kv}")
                    for kv in range(2)]
            for kv in range(2):
                nc.vector.memset(vext[kv][:], 1.0)

            def diag_off(jb, lc):
                """col offset of the diagonal 128-block inside chunk lc, or None"""
                od = 128 * jb - 512 * lc
                return od if 0 <= od < 512 else None

            q_tiles = {}

            def load_xt(lc):
                lsl = slice(lc * 512, (lc + 1) * 512)
                xt = []
                for db in range(16):
                    t = xt_p.tile([128, 512], f16, tag="xt", name=f"xt{db}")
                    nc.sync.dma_start(t[:], xT[db * 128:(db + 1) * 128, lsl])
                    xt.append(t)
                cos_t = maps_p.tile([128, 512], f32, tag="cos", name="cos_t")
                nc.sync.dma_start(cos_t[:], cosm_d[:, lsl])
                sin_t = maps_p.tile([128, 512], f32, tag="sin", name="sin_t")
                nc.sync.dma_start(sin_t[:], sinm2_d[:, lsl])
                return xt, cos_t, sin_t

            def do_proj(lc, loaded):
                lsl = slice(lc * 512, (lc + 1) * 512)
                xt, cos_t, sin_t = loaded

                def rope(ps, dest):
                    """rotate [128,512] psum block into dest (SBUF)."""
                    t2 = tmp_p.tile([128, 512], f32, tag="t2", bufs=2, name="t2")
                    nc.vector.tensor_tensor(t2[:], ps[:], sin_t[:], op=ALU.mult)
                    qsw = tmp_p.tile([128, 512], f32, tag="qsw", name="qsw")
                    for g, srcp in enumerate((32, 0, 96, 64)):
                        nc.sync.dma_start(qsw[g * 32:(g + 1) * 32, :],
                                          t2[srcp:srcp + 32, :])
                    t3 = tmp_p.tile([128, 512], f32, tag="t3", bufs=2, name="t3")
                    nc.vector.tensor_tensor(t3[:], ps[:], cos_t[:], op=ALU.mult)
                    nc.vector.tensor_tensor(dest[:], t3[:], qsw[:], op=ALU.add)

                # q projection + RoPE
                for cb in range(4):
                    ps = pj_ps.tile([128, 512], f32, tag="pj", name="ps_q")
                    for db in range(16):
                        nc.tensor.matmul(
                            ps[:], wq_sb[:, (cb * 16 + db) * 128:(cb * 16 + db + 1) * 128],
                            xt[db][:], start=(db == 0), stop=(db == 15))
                    qt = qrot_p.tile([128, 512], f16, name="qt")
                    rope(ps, qt)
                    q_tiles[(cb, lc)] = qt

                # k projection + RoPE + zero-padded scatter
                ps = pj_ps.tile([128, 512], f32, tag="pj", name="ps_k")
                for db in range(16):
                    nc.tensor.matmul(
                        ps[:], wk_sb[:, db * 128:(db + 1) * 128],
                        xt[db][:], start=(db == 0), stop=(db == 15))
                kraw = tmp_p.tile([128, 512], f16, tag="kraw", bufs=2, name="kraw")
                rope(ps, kraw)
                for kv in range(2):
                    nc.sync.dma_start(kz[kv][0][0:64, lsl],
                                      kraw[kv * 64:kv * 64 + 64, :])
                    nc.sync.dma_start(kz[kv][1][64:128, lsl],
                                      kraw[kv * 64:kv * 64 + 64, :])

                # v projection (transposed) then PE-transpose per block
                vt = tmp_p.tile([128, 512], f32, tag="vt", bufs=2, name="vt")
                ps = pj_ps.tile([128, 512], f32, tag="pj", name="ps_v")
                for db in range(16):
                    nc.tensor.matmul(
                        ps[:], wv_sb[:, db * 128:(db + 1) * 128],
                        xt[db][:], start=(db == 0), stop=(db == 15))
                nc.vector.tensor_copy(vt[:], ps[:])
                for j in range(4):
                    jb = 4 * lc + j
                    ps = pj_ps.tile([128, 128], f32, tag="pj", name="ps_t")
                    nc.tensor.transpose(ps[:], vt[:, j * 128:(j + 1) * 128],
                                        ident_sb[:])
                    for kv in range(2):
                        nc.vector.tensor_copy(
                            vext[kv][:, jb * 128:jb * 128 + 64],
                            ps[:, kv * 64:kv * 64 + 64])

            def do_att(lc):
                njb = 4 * lc + 4
                for t in range(4):
                    heads = (2 * t, 2 * t + 1)
                    aps_ = [at_ps.tile([128, 512], f32, tag="at", name=f"at{e}")
                            for e in range(2)]
                    P = None
                    for jb in range(njb):
                        o = max(0, 128 * jb - 512 * lc)
                        S = sc_ps.tile([128, 1024], f32, tag="sc", name="S")
                        for e, h in enumerate(heads):
                            kt = kz[h // 4][h % 2]
                            nc.tensor.matmul(
                                S[:, e * 512 + o:(e + 1) * 512],
                                kt[:, jb * 128:(jb + 1) * 128],
                                q_tiles[(h // 2, lc)][:, o:512],
                                start=True, stop=True)
                        od = diag_off(jb, lc)
                        if od is not None:
                            s3 = S.rearrange("p (e c) -> p e c", e=2)[:, :, od:od + 128]
                            m3 = mask_sb[:].unsqueeze(1).broadcast_to([128, 2, 128])
                            nc.vector.tensor_tensor(s3, s3, m3, op=ALU.add)
                        P = ptile_p.tile([128, 1024], f16, name="P")
                        if o == 0:
                            nc.scalar.activation(P[:], S[:], AF.Exp)
                        else:
                            s3 = S.rearrange("p (e c) -> p e c", e=2)[:, :, o:512]
                            p3 = P.rearrange("p (e c) -> p e c", e=2)[:, :, o:512]
                            nc.scalar.activation(p3, s3, AF.Exp)
                        for e, h in enumerate(heads):
                            kv = h // 4
                            nc.tensor.matmul(
                                aps_[e][:, o:512],
                                vext[kv][:, jb * 128:jb * 128 + 128],
                                P[:, e * 512 + o:(e + 1) * 512],
                                start=(jb == 0), stop=(jb == njb - 1),
                                skip_group_check=True)
                    if debug and lc == 0 and t == 0:
                        nc.sync.dma_start(dbg["p000"][:], P[:])
                    attT = attT_p.tile([128, 512], f16, name="attT")
                    q_tiles[("attT", lc, t)] = attT
                    if debug and lc == 0 and t == 0:
                        dbga = tmp_p.tile([128, 512], f32, tag="dbga", bufs=1,
                                          name="dbga")
                        nc.vector.tensor_copy(dbga[:], aps_[0][:])
                        nc.sync.dma_start(dbg["att0"][:], dbga[:])
                    for e, h in enumerate(heads):
                        den = rcpb_p.tile([1, 512], f32, tag="den", bufs=2,
                                          name="den")
                        nc.vector.tensor_copy(den[:], aps_[e][64:65, :])
                        rcpb = rcpb_p.tile([64, 512], f32, name="rcpb")
                        nc.vector.reciprocal_approx_fast(out=rcpb[0:1, :], in_=den[:])
                        nc.gpsimd.partition_broadcast(rcpb[:], rcpb[0:1, :])
                        if debug and lc == 0 and t == 0 and e == 0:
                            nc.sync.dma_start(dbg["rcpb0"][:], rcpb[:])
                        nc.vector.tensor_tensor(
                            attT[64 * e:64 * e + 64, :], aps_[e][0:64, :],
                            rcpb[:], op=ALU.mult)

            def do_outproj(lc):
                for lb in range(4):
                    for mc in range(4):
                        ps = pj_ps.tile([128, 512], f32, tag="pj", name="ps_o")
                        for cb in range(4):
                            nc.tensor.matmul(
                                ps[:],
                                q_tiles[("attT", lc, cb)][:, lb * 128:(lb + 1) * 128],
                                wo_sb[:, cb * 2048 + mc * 512:cb * 2048 + (mc + 1) * 512],
                                start=(cb == 0), stop=(cb == 3))
                        ysb = ysb_p.tile([128, 512], f32, name="ysb")
                        nc.scalar.copy(ysb[:], ps[:])
                        nc.sync.dma_start(
                            y_d[lc * 512 + lb * 128:lc * 512 + (lb + 1) * 128,
                                mc * 512:(mc + 1) * 512], ysb[:])

            ld = load_xt(0)
            load_weights_bulk()
            do_proj(0, ld)
            ld = load_xt(1)
            do_att(0)
            nc.sync.dma_start(wo_sb[:], wo_sb_d[:])
            do_proj(1, ld)
            ld = load_xt(2)
            do_outproj(0)
            do_att(1)
            do_proj(2, ld)
            ld = load_xt(3)
            do_outproj(1)
            do_att(2)
            do_proj(3, ld)
            do_outproj(2)
            do_att(3)
            do_outproj(3)

            if debug:
                nc.sync.dma_start(dbg["q00"][:], q_tiles[(0, 0)][:])
                nc.sync.dma_start(dbg["kz00"][:], kz[0][0][:])
                nc.sync.dma_start(dbg["vext0"][:], vext[0][:])
                nc.sync.dma_start(dbg["attT00"][:], q_tiles[("attT", 0, 0)][:])

    nc.compile()
    return nc


def _perm64(w):
    return np.concatenate([w[0::2], w[1::2]], axis=0)


def _prep_core_inputs(core, x, wq, wk, wv, wo, fc, fs, mask, xT_cache):
    b, g = divmod(core, 4)
    hq0 = 8 * g
    if b not in xT_cache:
        xT_cache[b] = np.ascontiguousarray(x[b].T, dtype=np.float16)
    xT = xT_cache[b]

    wq_s = (wq[hq0 * 64:(hq0 + 8) * 64] * SCALE).astype(F32)
    wq_p = np.concatenate([_perm64(wq_s[h * 64:(h + 1) * 64]) for h in range(8)], 0)
    wqT = wq_p.T  # [D, 512]
    # wq_sb[p, (cb*16+db)*128 + c] = wqT[db*128+p, cb*128+c]
    wq_sb = np.ascontiguousarray(
        wqT.reshape(16, 128, 4, 128).transpose(1, 2, 0, 3).reshape(128, 8192),
        dtype=np.float16)

    wk_s = wk[2 * g * 64:(2 * g + 2) * 64]
    wk_p = np.concatenate([_perm64(wk_s[h * 64:(h + 1) * 64]) for h in range(2)], 0)
    wkT = wk_p.T  # [D, 128]
    wk_sb = np.ascontiguousarray(
        wkT.reshape(16, 128, 128).transpose(1, 0, 2).reshape(128, 2048),
        dtype=np.float16)

    wvT = wv[2 * g * 64:(2 * g + 2) * 64].T  # [D, 128]
    wv_sb = np.ascontiguousarray(
        wvT.reshape(16, 128, 128).transpose(1, 0, 2).reshape(128, 2048),
        dtype=np.float16)

    woT = wo[:, hq0 * 64:(hq0 + 8) * 64].T  # [512, D]
    wo_sb = np.ascontiguousarray(
        woT.reshape(4, 128, 4, 512).transpose(1, 0, 2, 3).reshape(128, 8192),
        dtype=np.float16)

    cosT = np.tile(fc.T, (4, 1)).astype(F32)  # [128, L]
    sinT = np.tile(fs.T, (4, 1)).astype(F32)
    sgn = np.ones((128, 1), F32)
    sgn[32:64] = -1
    sgn[96:128] = -1
    sinm2 = np.ascontiguousarray(sinT * sgn)

    maskT = np.ascontiguousarray(mask[0, 0, :128, :128].T, dtype=F32)

    return {"xT": xT, "wq_sb": wq_sb, "wk_sb": wk_sb, "wv_sb": wv_sb,
            "wo_sb": wo_sb, "cosm": np.ascontiguousarray(cosT),
            "sinm2": sinm2, "maskT": maskT,
            "ident": np.eye(128, dtype=F32)}


def kernel(x, wq, wk, wv, wo, freqs_cos, freqs_sin, mask):
    from concourse import bass_utils

    if "nc" not in _CACHE:
        _CACHE["nc"] = _build_nc()
    nc = _CACHE["nc"]

    x = np.asarray(x, F32)
    xT_cache = {}
    in_maps = [
        _prep_core_inputs(c, x, np.asarray(wq, F32), np.asarray(wk, F32),
                          np.asarray(wv, F32), np.asarray(wo, F32),
                          np.asarray(freqs_cos, F32), np.asarray(freqs_sin, F32),
                          np.asarray(mask, F32), xT_cache)
        for c in range(NCORES)
    ]
    res = bass_utils.run_bass_kernel_spmd(nc, in_maps, core_ids=list(range(NCORES)))
    out = np.zeros((B, L, D), F32)
    for c in range(NCORES):
        out[c // 4] += res.results[c]["y"]
    return out
